# revision 7
# baseline (speedup 1.0000x reference)
"""Multi-head causal+padded attention on 8 TRN2 NeuronCores.

Strategy: data-parallel over batch (8 batches -> 8 cores, no collectives).
Per core, everything is computed in a transposed layout so that no PE
transposes of the attention matrix are needed, and the Q/V projections are
folded away algebraically:

  scores  = (q Wq^T)(k Wk^T)^T = q (Wq^T Wk) k^T
            -> G[h] = (Wk_h^T Wq_h)^T-matmul over kT   [e, tk]  (per head)
            -> S^T(kc,:) = G[h][:,kc]^T-block @ qT     [tk-part, tq-free]
  A^T     = exp(s * S^T)                   (key-pad mask via zeroed kT rows,
                                            causal diag via idb x tri matmul
                                            injected into PSUM)
  rowsum  = mkw^T @ A^T  (+ 65504*caserow outer product for degenerate rows)
  P[h]    = sum_kc k_nat[kc]^T-block @ A^T      [e, tq]   (raw masked keys!)
  Pn[h]   = P[h] * recip(rowsum)
  out^T   = sum_h (Wu_h Wv_h)^T-block @ Pn[h] + w2^T @ brows + bu

so the only PSUM->SBUF evacuations are the 8 G[h] tiles and the epilogue
multiplies. S^T is stored bf16 in PSUM (1024/bank) so pairs of aligned key
blocks share one bank and one wide Exp (the ~260ns/op ACT overhead is the
scalar bottleneck); A^T stays f16. All PE operands are f16/bf16 (host
pre-rounds; f32 accumulate), the four big input DMAs ride four parallel
queues, a few throwaway matmuls into the fin banks warm the PE clock-gate
during the DMA window, and the whole kernel runs as one interleaved stream
(G[h+1] projected inside head h's attention; output projection accumulates
into a persistent PSUM pair as each head's Pn half completes). Degenerate
softmax rows are fixed exactly via a rank-2 correction (w2/brows).
"""

import numpy as np

import concourse.bacc as bacc
import concourse.mybir as mybir
import concourse.tile as tile
from concourse.bass_utils import run_bass_kernel_spmd

F32 = mybir.dt.float32
F16 = mybir.dt.float16
BF16 = mybir.dt.bfloat16

B, TQ, TK, E, H = 8, 1024, 1024, 128, 8
HE = H * E
SCALE = float(E) ** -0.5
TRI_NEG = -60000.0
CASE_BIG = 65504.0


def _build():
    nc = bacc.Bacc("TRN2", target_bir_lowering=False, debug=False)
    dp = nc.declare_dram_parameter
    d_qT = dp("qT", [E, TQ], F16, isOutput=False)
    d_kT = dp("kT", [E, TK], F16, isOutput=False)
    d_kn = dp("kn", [TK, E], F16, isOutput=False)
    d_mT = dp("mT", [E, HE], F16, isOutput=False)
    d_nuT = dp("nuT", [HE, E], F16, isOutput=False)
    d_mkw = dp("mkw", [TK, 128], F16, isOutput=False)
    d_tri = dp("trineg", [128, 128], F16, isOutput=False)
    d_idb = dp("identb", [128, 128], F16, isOutput=False)
    d_case = dp("caserow", [1, TQ], F16, isOutput=False)
    d_onesc = dp("onesc", [1, 128], F16, isOutput=False)
    d_brow = dp("brows", [2, TQ], F16, isOutput=False)
    d_w2 = dp("w2", [2, E], F16, isOutput=False)
    d_bu = dp("bu", [E, 1], F32, isOutput=False)
    d_out = dp("out", [E, TQ], F32, isOutput=True)

    Exp = mybir.ActivationFunctionType.Exp
    Ident = mybir.ActivationFunctionType.Identity
    mult = mybir.AluOpType.mult
    mm = nc.tensor.matmul

    with tile.TileContext(nc) as tc:
        with (
            tc.tile_pool(name="const", bufs=1) as cp,
            tc.tile_pool(name="persist", bufs=1) as pp,
        ):
            # ---- input DMAs: one per queue so they land in parallel ----
            kTs = cp.tile([E, TK], F16, tag="kTs", name="kTs")
            mall = cp.tile([E, HE], F16, tag="mall", name="mall")
            qTs = cp.tile([E, TQ], F16, tag="qTs", name="qTs")
            knall = cp.tile([128, HE], F16, tag="knall", name="knall")
            # split + ordered by first use: G0 needs kT halves + mall[:,0:128];
            # the first attention unit is (0, half=1) so it reads qT[512:]
            nc.sync.dma_start(out=kTs[:, 0:512], in_=d_kT[:, 0:512])
            nc.scalar.dma_start(out=mall[:, 0:256], in_=d_mT[:, 0:256])
            nc.sync.dma_start(out=kTs[:, 512:TK], in_=d_kT[:, 512:TK])
            nc.scalar.dma_start(out=qTs[:, 512:TQ], in_=d_qT[:, 512:TQ])
            nc.scalar.dma_start(out=qTs[:, 0:512], in_=d_qT[:, 0:512])
            nc.sync.dma_start(
                out=knall[:].rearrange("p (c e) -> p c e", c=8),
                in_=d_kn.rearrange("(c p) e -> p c e", p=128),
            )
            nc.scalar.dma_start(out=mall[:, 256:HE], in_=d_mT[:, 256:HE])
            kn = [knall[:, kc * 128 : (kc + 1) * 128] for kc in range(8)]

            # ---- constants (gpsimd queue; attention consts first) ----
            tri = cp.tile([128, 128], F16, tag="tri", name="tri")
            nc.gpsimd.dma_start(out=tri[:], in_=d_tri[:])
            idb = cp.tile([128, 128], F16, tag="idb", name="idb")
            nc.gpsimd.dma_start(out=idb[:], in_=d_idb[:])
            mkwall = cp.tile([128, HE], F16, tag="mkwall", name="mkwall")
            nc.gpsimd.dma_start(
                out=mkwall[:].rearrange("p (c e) -> p c e", c=8),
                in_=d_mkw.rearrange("(c p) e -> p c e", p=128),
            )
            mkw = [mkwall[:, kc * 128 : (kc + 1) * 128] for kc in range(8)]
            case = cp.tile([1, TQ], F16, tag="case", name="case")
            nc.gpsimd.dma_start(out=case[:], in_=d_case[:])
            onesc = cp.tile([1, 128], F16, tag="onesc", name="onesc")
            nc.gpsimd.dma_start(out=onesc[:], in_=d_onesc[:])
            nuall = cp.tile([128, HE], F16, tag="nuall", name="nuall")
            nc.gpsimd.dma_start(
                out=nuall[:].rearrange("p (c e) -> p c e", c=8),
                in_=d_nuT.rearrange("(c p) e -> p c e", p=128),
            )
            nu = [nuall[:, h * 128 : (h + 1) * 128] for h in range(H)]
            brow = cp.tile([2, TQ], F16, tag="brow", name="brow")
            nc.gpsimd.dma_start(out=brow[:], in_=d_brow[:])
            w2 = cp.tile([2, 128], F16, tag="w2", name="w2")
            nc.gpsimd.dma_start(out=w2[:], in_=d_w2[:])
            bu = cp.tile([E, 1], F32, tag="bu", name="bu")
            nc.gpsimd.dma_start(out=bu[:], in_=d_bu[:])

            # ---- exp table preload (hide ~2.7us ACT_TABLE_LOAD) ----
            dmy = cp.tile([128, 1], F32, tag="dmy", name="dmy")
            dmyo = cp.tile([128, 1], F32, tag="dmyo", name="dmyo")
            nc.vector.memset(dmy[:], 0.0)
            nc.scalar.activation(out=dmyo[:], in_=dmy[:], func=Exp,
                                 bias=0.0, scale=1.0)
            # zero tile for PE warm-up matmuls (also during DMA window)
            zs = cp.tile([128, 512], F16, tag="zs", name="zs")
            nc.vector.memset(zs[:], 0.0)

            # ---- persistent activations ----
            G = [pp.tile([128, TK], F16, tag=f"G{h}", name=f"G{h}")
                 for h in range(H)]
            Pn = [pp.tile([128, TQ], F16, tag=f"Pn{h}", name=f"Pn{h}")
                  for h in range(H)]

            with (
                tc.tile_pool(name="stps", bufs=2, space="PSUM") as sp,
                tc.tile_pool(name="accps", bufs=2, space="PSUM") as ap_,
                tc.tile_pool(name="finps", bufs=1, space="PSUM") as fp_,
                tc.tile_pool(name="atp", bufs=6) as atp,
                tc.tile_pool(name="ssp", bufs=2) as ssp,
            ):
                n_evac = 0

                def evac(dst, src):
                    # mostly vector; scalar takes every 4th chunk (its queue
                    # must stay clear for the exp stream)
                    nonlocal n_evac
                    if n_evac % 4 == 3:
                        nc.scalar.copy(dst, src)
                    else:
                        nc.vector.tensor_copy(dst, src)
                    n_evac += 1

                def proj_g(h):
                    for i, (a, b) in enumerate(((0, 512), (512, TK))):
                        ps = sp.tile([128, 512], F32, tag="st",
                                     name=f"psg{h}_{i}")
                        mm(ps[:], mall[:, h * 128 : (h + 1) * 128],
                           kTs[:, a:b], start=True, stop=True)
                        evac(G[h][:, a:b], ps[:])

                fin = [fp_.tile([128, 512], F32, tag=f"fin{i}",
                                name=f"fin{i}") for i in range(2)]

                # PE warm-up: throwaway matmuls into the fin banks (later
                # wiped by the first start=True accumulation)
                for i in range(6):
                    mm(fin[i % 2][:], zs[:, 0:128], zs[:],
                       start=True, stop=True)

                # units: long half first so the kernel tail is a short unit
                units = [(h, half) for h in range(H) for half in (1, 0)]
                state = {}
                fin_started = [False, False]
                fin_last = {}
                for u, (h, half) in enumerate(units):
                    fin_last[half] = u

                def emit_epilogue(u):
                    h, half = units[u]
                    q0 = half * 512
                    sum_ps, out_ps = state[u]
                    rb = ssp.tile([128, 512], F32, tag="rb", name=f"rb{u}")
                    nc.vector.reciprocal_approx_fast(out=rb[:], in_=sum_ps[:])
                    nc.vector.tensor_tensor(
                        out=Pn[h][:, q0 : q0 + 512], in0=out_ps[:],
                        in1=rb[:], op=mult,
                    )

                def emit_fin(u):
                    h, half = units[u]
                    q0 = half * 512
                    mm(fin[half][:], nu[h][:], Pn[h][:, q0 : q0 + 512],
                       start=not fin_started[half],
                       stop=(u == fin_last[half]))
                    fin_started[half] = True

                def emit_w2(half):
                    q0 = half * 512
                    mm(fin[half][:], w2[:], brow[:, q0 : q0 + 512],
                       start=not fin_started[half], stop=False)
                    fin_started[half] = True

                proj_g(0)

                for u, (h, half) in enumerate(units):
                    q0 = half * 512
                    kcs = range(4) if half == 0 else range(8)
                    klast = kcs[-1]
                    sum_ps = ap_.tile([128, 512], F32, tag="sum_ps",
                                      name=f"sum{u}")
                    out_ps = ap_.tile([128, 512], F32, tag="out_ps",
                                      name=f"out{u}")
                    state[u] = (sum_ps, out_ps)
                    ats = {}

                    def consume(kc, sum_ps=sum_ps, out_ps=out_ps, ats=ats,
                                q0=q0, klast=klast):
                        r0 = max(kc * 128 - q0, 0)
                        n = 512 - r0
                        at_t, off = ats[kc]
                        mm(sum_ps[:, r0:512], mkw[kc][:],
                           at_t[:, off : off + n], start=(kc == 0),
                           stop=False)
                        mm(out_ps[:, r0:512], kn[kc][:],
                           at_t[:, off : off + n], start=(kc == 0),
                           stop=(kc == klast))

                    def emit_interleaves(u, h, half, kc):
                        if kc >= 3:
                            consume(kc - 3)
                        if kc == 3 and u >= 1:
                            emit_epilogue(u - 1)
                            if u >= 2:
                                emit_fin(u - 2)
                        if half == 1:
                            if kc == 2 and h < H - 1:
                                proj_g(h + 1)
                            elif kc == 5 and h == 0:
                                emit_w2(0)
                                emit_w2(1)

                    for kc in kcs:
                        r0 = max(kc * 128 - q0, 0)
                        n = 512 - r0
                        diag = q0 <= kc * 128 < q0 + 512
                        st = sp.tile([128, 512], F32, tag="st",
                                     name=f"st{u}_{kc}")
                        mm(st[:, r0:512], G[h][:, kc * 128 : (kc + 1) * 128],
                           qTs[:, q0 + r0 : q0 + 512], start=True,
                           stop=not diag)
                        if diag:
                            mm(st[:, r0 : r0 + 128], idb[:], tri[:],
                               start=False, stop=True)
                        at = atp.tile([128, 512], F16, tag="at",
                                      name=f"at{u}_{kc}")
                        ats[kc] = (at, 0)
                        nc.scalar.activation(
                            out=at[:, 0:n], in_=st[:, r0:512], func=Exp,
                            bias=0.0, scale=SCALE,
                        )
                        emit_interleaves(u, h, half, kc)
                    for t in (2, 1, 0):
                        if klast >= t:
                            consume(klast - t)
                    mm(sum_ps[:], onesc[:], case[:, q0 : q0 + 512],
                       start=False, stop=True)

                emit_epilogue(len(units) - 1)
                outsb = pp.tile([E, TQ], F32, tag="outsb", name="outsb")
                for u in (len(units) - 2, len(units) - 1):
                    emit_fin(u)
                    half = units[u][1]
                    a = half * 512
                    nc.scalar.activation(
                        out=outsb[:, a : a + 512], in_=fin[half][:],
                        func=Ident, bias=bu[:, 0:1], scale=1.0,
                    )
                    nc.sync.dma_start(out=d_out[:, a : a + 512],
                                      in_=outsb[:, a : a + 512])

    nc.compile()
    return nc


_NC = None


def _get_nc():
    global _NC
    if _NC is None:
        _NC = _build()
    return _NC


def _host_prep(q, k, mask_q, mask_k, Wq, Wk, Wv, Wu, bu):
    f16 = np.float16
    # fold projections: scores = q (Wq^T Wk) k^T ; out = (Wu_h Wv_h) (k^T A)
    mT = np.concatenate(
        [Wk[h * E : (h + 1) * E].T @ Wq[h * E : (h + 1) * E]
         for h in range(H)], axis=1)  # [E, H*E], col block h = Wk_h^T Wq_h
    nuT = np.concatenate(
        [(Wu[:, h * E : (h + 1) * E] @ Wv[h * E : (h + 1) * E]).T
         for h in range(H)], axis=0)  # [H*E, E], row block h = (Wu_h Wv_h)^T
    shared = {
        "mT": np.ascontiguousarray(mT).astype(f16),
        "nuT": np.ascontiguousarray(nuT).astype(f16),
        "trineg": (TRI_NEG * np.tril(np.ones((128, 128), np.float32), -1)
                   ).astype(f16),
        "identb": np.eye(128).astype(f16),
        "onesc": np.full((1, 128), CASE_BIG, f16),
        "bu": np.ascontiguousarray(bu[:, None]).astype(np.float32),
    }
    WuWv = (Wu @ Wv).astype(np.float32)
    in_maps = []
    for b in range(B):
        mq = mask_q[b, :, 0].astype(np.float32)
        mk = mask_k[b, :, 0].astype(np.float32)
        c01 = (np.cumsum(mk) >= 1.0).astype(np.float32)
        caseA = mq * c01
        b1 = mq * (1.0 - c01)
        b2 = 1.0 - mq
        s1m = 1.0 - mk
        denom = max(float(s1m.sum()), 1.0)
        wvecs = np.stack([s1m / denom, np.full(TK, 1.0 / TK, np.float32)],
                         axis=1)
        w2 = (wvecs.T.astype(np.float32) @ k[b]) @ WuWv.T
        km = k[b] * mk[:, None]
        m = dict(shared)
        m["qT"] = np.ascontiguousarray(q[b].T).astype(f16)
        m["kT"] = np.ascontiguousarray(km.T).astype(f16)
        m["kn"] = np.ascontiguousarray(km).astype(f16)
        m["mkw"] = np.ascontiguousarray(
            np.broadcast_to(mk[:, None], (TK, 128))
        ).astype(f16)
        m["caserow"] = (CASE_BIG * (1.0 - caseA))[None, :].astype(f16)
        m["brows"] = np.stack([b1, b2]).astype(f16)
        m["w2"] = np.ascontiguousarray(w2).astype(f16)
        in_maps.append(m)
    return in_maps


def kernel(q, k, mask_q, mask_k, Wq, Wk, Wv, Wu, bu):
    nc = _get_nc()
    in_maps = _host_prep(q, k, mask_q, mask_k, Wq, Wk, Wv, Wu, bu)
    res = run_bass_kernel_spmd(nc, in_maps, list(range(B)))
    out = np.stack([np.ascontiguousarray(res.results[b]["out"].T)
                    for b in range(B)])
    return out.astype(np.float32)
